# revision 8
# baseline (speedup 1.0000x reference)
"""NeRF MLP forward on 8 Trainium2 NeuronCores (Bass/Tile), data-parallel.

Layout: feature-major ([features, points]) so the MLP chains through the
PE array with weights as the stationary operand (lhsT = W[fan_in, fan_out]).

Positional encoding: args = x * (2^l / 2pi) + (0 | 0.25) are range-reduced
with a custom DVE op (frac-center via the +-1.5*2^23 magic-number trick) in a
single instruction, then evaluated with one ACT Sin op per block
(sin(2pi f) covers both sin and cos rows; the quarter-turn shift is applied
before reduction so the argument stays within the LUT domain [-pi, pi]).

Algebraic folds (host-side, exact):
  - t = a22 @ w23 + b23 has no activation on its first 128 cols, so
    W3' = w23c @ w3a and b3' = b23c @ w3a + b3 fold layer 23c away.
  - sigma shares rhs a22 with W3', so it rides as column 64 of an M=65
    matmul; one relu evac produces h9 and final sigma together.
  - final sigmoid is applied on host to the DMA'd logits (the Sin table set
    has no sigmoid and a per-tile ACT table switch costs ~2.7us).
"""
import os
import sys
import numpy as np

NC = 8
N_TOTAL = 524288
NPC = N_TOTAL // NC  # 65536 points per core
HID = 128
L_POS = 10
L_DIR = 4
JB = 4096    # emb batch (bcast/frac/sin granularity)
JS = 1024    # evac sub-tile granularity
TM = 512     # matmul free-dim tile
TWO_PI = 2.0 * np.pi
MAGIC = 12582912.0  # 1.5 * 2**23  (fp32 round-to-nearest-int trick)

FRAC_SHA_V3 = "3d790cc1ec454799"

_cache = {}


def _build_frac_op():
    """out = a - round(a), a = Src0*C0 + C1  (one DVE pass)."""
    from concourse import dve_ops
    from concourse.dve_ops import DveOp
    from concourse.dve_spec import Spec, Src0, C0, C1, C2

    name = "FRAC_CENTER"
    for o in dve_ops.OPS:
        if o.name == name:
            return o

    def make(sha):
        _a = Src0 * C0 + C1
        return DveOp(
            name,
            Spec(
                body=_a - ((_a + C2) - C2),
                reference=lambda in0, in1, c0, c1, c2: (
                    lambda a: (a - ((a + np.float32(c2)) - np.float32(c2))).astype(np.float32)
                )(np.float32(in0 * c0 + c1)),
            ),
            subdim=False,
            uops_sha={"v3": sha},
        )

    op = make(FRAC_SHA_V3)
    dve_ops.OPS.append(op)
    dve_ops._SUB_OPCODE_FOR_NAME[name] = dve_ops._CUSTOM_DVE_ROW_BASE + len(dve_ops.OPS) - 1
    dve_ops.CUSTOM_DVE_SPECS[name] = op.spec
    try:
        op.compile("v3")
    except ValueError as e:
        # sha drifted with toolchain; re-pin from the error message
        import re
        m = re.search(r"v3: (\w+)", str(e))
        dve_ops.OPS.remove(op)
        op = make(m.group(1))
        dve_ops.OPS.append(op)
        dve_ops.CUSTOM_DVE_SPECS[name] = op.spec
        op.compile("v3")
    return op


def _emb_perm(n_l):
    """Device emb row j -> reference feature row. Rows: 3*n_l sin (l-major),
    3*n_l cos, then x,y,z passthrough."""
    perm = []
    for rep in range(2 * n_l):
        l = rep % n_l
        t = 0 if rep < n_l else 1
        for ax in range(3):
            perm.append(3 + 6 * l + 3 * t + ax)
    perm += [0, 1, 2]
    return np.array(perm)


def _build(weights):
    """Build + compile the Bass program. weights: dict of prepped np arrays."""
    import concourse.bass as bass
    import concourse.tile as tile
    from concourse import bacc, mybir
    from contextlib import ExitStack

    F32 = mybir.dt.float32
    F32R = mybir.dt.float32r
    AF = mybir.ActivationFunctionType
    ALU = mybir.AluOpType

    frac_op = _build_frac_op()

    nc = bacc.Bacc("TRN2", target_bir_lowering=False, debug=False,
                   enable_asserts=False, num_devices=1)

    od_ap = nc.dram_tensor("od", [6, NPC], F32, kind="ExternalInput").ap()
    sv_ap = nc.dram_tensor("sv", [88, 2], F32, kind="ExternalInput").ap()
    bias_ap = nc.dram_tensor("bias", [128, 16], F32, kind="ExternalInput").ap()
    w_aps = {}
    for name in ("w10", "w11", "w12", "w13", "w20a", "w20b", "w21", "w22",
                 "w3e", "w3be", "wc"):
        w_aps[name] = nc.dram_tensor(name, list(weights[name].shape), F32R,
                                     kind="ExternalInput").ap()
    outc_ap = nc.dram_tensor("outc", [3, NPC], F32, kind="ExternalOutput").ap()
    outs_ap = nc.dram_tensor("outs", [1, NPC], F32R, kind="ExternalOutput").ap()

    # bias column indices
    BI = {"b10": 0, "b11": 1, "b12": 2, "b13": 3, "b20": 4, "b21": 5,
          "b22": 6, "b3e": 7, "bc": 8}

    with tile.TileContext(nc) as tc, ExitStack() as ctx:
        cpool = ctx.enter_context(tc.tile_pool(name="const", bufs=1))
        embp = ctx.enter_context(tc.tile_pool(name="emb", bufs=2))
        hp = ctx.enter_context(tc.tile_pool(name="h", bufs=4))
        h9p = ctx.enter_context(tc.tile_pool(name="h9", bufs=2))
        c3p = ctx.enter_context(tc.tile_pool(name="c3", bufs=2))
        psb = ctx.enter_context(tc.tile_pool(name="psb", bufs=3, space="PSUM"))
        pss = ctx.enter_context(tc.tile_pool(name="pss", bufs=1, space="PSUM"))

        wt = {}
        for name, ap in w_aps.items():
            wt[name] = cpool.tile(list(weights[name].shape), F32R, tag=name, name="wt_" + name)
            nc.sync.dma_start(wt[name][:], ap[:])
        svt = cpool.tile([88, 2], F32, tag="sv")
        nc.sync.dma_start(svt[:], sv_ap[:])
        bt = cpool.tile([128, 16], F32, tag="bias")
        nc.sync.dma_start(bt[:], bias_ap[:])

        def bias(name, rows=128):
            return bt[0:rows, BI[name]:BI[name] + 1]

        def relu_evac(eng, dst, src, bname, rows=128):
            if eng == "act":
                nc.scalar.activation(dst, src, AF.Relu, bias=bias(bname, rows))
            else:
                nc.vector.tensor_scalar(dst, src, bias(bname, rows), 0.0,
                                        ALU.add, ALU.max)

        n_batches = NPC // JB
        for b in range(n_batches):
            c0 = b * JB
            # ---- embedding stage (JB-wide) ----
            xb = embp.tile([88, JB], F32, tag="xb")
            src_pos = bass.AP(tensor=od_ap.tensor, offset=c0,
                              ap=[[0, 20], [NPC, 3], [1, JB]])
            nc.sync.dma_start(xb[0:60, :], src_pos)
            src_fill = bass.AP(tensor=od_ap.tensor, offset=c0,
                               ap=[[0, 4], [1, JB]])
            nc.sync.dma_start(xb[60:64, :], src_fill)
            src_dir = bass.AP(tensor=od_ap.tensor, offset=3 * NPC + c0,
                              ap=[[0, 8], [NPC, 3], [1, JB]])
            nc.sync.dma_start(xb[64:88, :], src_dir)

            f = embp.tile([88, JB], F32, tag="f")
            nc.vector._custom_dve(frac_op, out=f[:], in0=xb[:], in1=None,
                                  s0=svt[:, 0:1], s1=svt[:, 1:2], imm2=MAGIC)

            ex = embp.tile([63, JB], F32R, tag="ex")
            nc.scalar.activation(ex[0:60, :], f[0:60, :], AF.Sin, scale=TWO_PI)
            nc.gpsimd.dma_start(ex[60:63, :], od_ap[0:3, c0:c0 + JB])
            ed = embp.tile([27, JB], F32R, tag="ed")
            nc.scalar.activation(ed[0:24, :], f[64:88, :], AF.Sin, scale=TWO_PI)
            nc.gpsimd.dma_start(ed[24:27, :], od_ap[3:6, c0:c0 + JB])

            for j in range(JB // JS):
                jc = j * JS
                nsub = JS // TM

                def layer(wname, rhs_tile, rhs_rows, rhs_col0, bname, eng,
                          out_rows=128, extra=None):
                    """matmul (+optional K-split extra) then relu evac.
                    rhs_col0: column base of this subtile within rhs_tile."""
                    ps = psb.tile([out_rows, JS], mybir.dt.float32, tag="big", name="ps_" + wname)
                    for s in range(nsub):
                        rs = slice(rhs_col0 + s * TM, rhs_col0 + (s + 1) * TM)
                        nc.tensor.matmul(ps[:, s * TM:(s + 1) * TM],
                                         wt[wname][:], rhs_tile[0:rhs_rows, rs],
                                         start=True, stop=not extra)
                        if extra:
                            ew, et, er, ec0 = extra
                            es = slice(ec0 + s * TM, ec0 + (s + 1) * TM)
                            nc.tensor.matmul(ps[:, s * TM:(s + 1) * TM],
                                             wt[ew][:], et[0:er, es],
                                             start=False, stop=True)
                    h = (h9p if out_rows == 65 else hp).tile(
                        [out_rows, JS], F32R, tag="h9" if out_rows == 65 else "h",
                        name="h_" + wname)
                    relu_evac(eng, h[:], ps[:], bname, out_rows)
                    return h

                h1 = layer("w10", ex, 63, jc, "b10", "act")
                h2 = layer("w11", h1, 128, 0, "b11", "dve")
                h3 = layer("w12", h2, 128, 0, "b12", "act")
                h4 = layer("w13", h3, 128, 0, "b13", "dve")
                h5 = layer("w20a", h4, 128, 0, "b20", "dve",
                           extra=("w20b", ex, 63, jc))
                h6 = layer("w21", h5, 128, 0, "b21", "act")
                a22 = layer("w22", h6, 128, 0, "b22", "dve")
                h9e = layer("w3e", a22, 128, 0, "b3e", "dve", out_rows=65,
                            extra=("w3be", ed, 27, jc))

                # sigma: row 64 of h9e, already relu'd + biased
                nc.sync.dma_start(outs_ap[0:1, c0 + jc:c0 + jc + JS],
                                  h9e[64:65, :])

                # c logits: wc [64,3] @ h9e[0:64]
                psc = pss.tile([3, JS], mybir.dt.float32, tag="small")
                for s in range(nsub):
                    nc.tensor.matmul(psc[:, s * TM:(s + 1) * TM], wt["wc"][:],
                                     h9e[0:64, s * TM:(s + 1) * TM],
                                     start=True, stop=True)
                c3 = c3p.tile([3, JS], F32, tag="c3")
                nc.scalar.activation(c3[:], psc[:], AF.Identity, bias=bias("bc", 3))
                nc.sync.dma_start(outc_ap[:, c0 + jc:c0 + jc + JS], c3[:])

    nc.compile()
    return nc


def _prep(inputs):
    """Host-side weight folding and constant prep (float64 folds, f32 out)."""
    f64 = {k: np.asarray(v, np.float64) for k, v in inputs.items()
           if k not in ("o", "d")}
    perm_x = _emb_perm(L_POS)   # 63
    perm_d = _emb_perm(L_DIR)   # 27

    w10 = f64["w10"][perm_x]
    w20 = f64["w20"]
    w20a, w20b = w20[:128], w20[128:][perm_x]
    w23 = f64["w23"]
    w23c, w23s = w23[:, :128], w23[:, 128]
    b23 = f64["b23"]
    b23c, b23s = b23[:128], b23[128]
    w3 = f64["w3"]
    w3a, w3b = w3[:128], w3[128:][perm_d]

    w3p = w23c @ w3a                       # [128, 64]
    b3p = b23c @ w3a + f64["b3"]           # [64]
    w3e = np.concatenate([w3p, w23s[:, None]], axis=1)        # [128, 65]
    w3be = np.concatenate([w3b, np.zeros((27, 1))], axis=1)   # [27, 65]
    b3e = np.concatenate([b3p, [b23s]])                       # [65]

    W = dict(
        w10=w10, w11=f64["w11"], w12=f64["w12"], w13=f64["w13"],
        w20a=w20a, w20b=w20b, w21=f64["w21"], w22=f64["w22"],
        w3e=w3e, w3be=w3be, wc=f64["w4"],
    )
    # PE reads truncate fp32 operands to FP22 (toward zero): each w*x product
    # is low by ~2*2^-12 in expectation; compensate with a weight pre-scale.
    comp = 1.0 + float(os.environ.get("KERNEL_FP22_COMP", "0"))
    W = {k: np.ascontiguousarray(v * comp, dtype=np.float32) for k, v in W.items()}

    bias = np.zeros((128, 16), np.float32)
    for i, k in enumerate(("b10", "b11", "b12", "b13", "b20", "b21", "b22")):
        bias[:, i] = f64[k].astype(np.float32)
    bias[0:65, 7] = b3e.astype(np.float32)
    bias[0:3, 8] = f64["b4"].astype(np.float32)

    sv = np.zeros((88, 2), np.float32)
    for rep in range(20):
        l = rep % 10
        sv[rep * 3:rep * 3 + 3, 0] = (2.0 ** l) / TWO_PI
        sv[rep * 3:rep * 3 + 3, 1] = 0.0 if rep < 10 else 0.25
    for rep in range(8):
        l = rep % 4
        r0 = 64 + rep * 3
        sv[r0:r0 + 3, 0] = (2.0 ** l) / TWO_PI
        sv[r0:r0 + 3, 1] = 0.0 if rep < 4 else 0.25

    return W, bias, sv


def _run(inputs, trace=False, trace_kwargs=None):
    from concourse.bass_utils import run_bass_kernel_spmd

    W, bias, sv = _prep(inputs)
    key = "nc"
    if key not in _cache:
        _cache[key] = _build(W)
    nc = _cache[key]

    o = np.asarray(inputs["o"], np.float32)
    d = np.asarray(inputs["d"], np.float32)
    oT = np.ascontiguousarray(o.T)  # [3, N]
    dT = np.ascontiguousarray(d.T)

    in_maps = []
    for c in range(NC):
        cs, ce = c * NPC, (c + 1) * NPC
        od = np.ascontiguousarray(np.concatenate([oT[:, cs:ce], dT[:, cs:ce]], axis=0))
        m = dict(od=od, sv=sv, bias=bias)
        for k, v in W.items():
            m[k] = v
        in_maps.append(m)

    kw = {}
    if trace:
        kw["trace"] = True
        if trace_kwargs:
            kw.update(trace_kwargs)
    res = run_bass_kernel_spmd(nc, in_maps, core_ids=list(range(NC)), **kw)

    c_parts, s_parts = [], []
    for cr in range(NC):
        logits = res.results[cr]["outc"]           # [3, NPC]
        sig = res.results[cr]["outs"][0]           # [NPC]
        c_parts.append(0.5 * (1.0 + np.tanh(0.5 * logits.astype(np.float64))))
        s_parts.append(sig)
    c = np.ascontiguousarray(np.concatenate(c_parts, axis=1).T.astype(np.float32))
    sigma = np.concatenate(s_parts).astype(np.float32)
    return (c, sigma), res


def kernel(**inputs):
    (c, sigma), _ = _run(inputs)
    return c, sigma


# revision 9
# speedup vs baseline: 1.1595x; 1.1595x over previous
"""NeRF MLP forward on 8 Trainium2 NeuronCores (Bass/Tile), data-parallel.

Layout: feature-major ([features, points]) so the MLP chains through the
PE array with weights as the stationary operand (lhsT = W[fan_in, fan_out]).

Positional encoding: args = x * (2^l / 2pi) + (0 | 0.25) are range-reduced
with a custom DVE op (frac-center via the +-1.5*2^23 magic-number trick) in a
single instruction, then evaluated with one ACT Sin op per block
(sin(2pi f) covers both sin and cos rows; the quarter-turn shift is applied
before reduction so the argument stays within the LUT domain [-pi, pi]).

Algebraic folds (host-side, exact):
  - t = a22 @ w23 + b23 has no activation on its first 128 cols, so
    W3' = w23c @ w3a and b3' = b23c @ w3a + b3 fold layer 23c away.
  - sigma shares rhs a22 with W3', so it rides as column 64 of an M=65
    matmul; one relu evac produces h9 and final sigma together.
  - final sigmoid is applied on host to the DMA'd logits (the Sin table set
    has no sigmoid and a per-tile ACT table switch costs ~2.7us).
"""
import os
import sys
import numpy as np

NC = 8
N_TOTAL = 524288
NPC = N_TOTAL // NC  # 65536 points per core
HID = 128
L_POS = 10
L_DIR = 4
JB = 2048    # emb batch (bcast/frac/sin granularity)
JS = 1024    # evac sub-tile granularity
TM = 512     # matmul free-dim tile
TWO_PI = 2.0 * np.pi
MAGIC = 12582912.0  # 1.5 * 2**23  (fp32 round-to-nearest-int trick)

FRAC_SHA_V3 = "3d790cc1ec454799"

_cache = {}


def _build_frac_op():
    """out = a - round(a), a = Src0*C0 + C1  (one DVE pass)."""
    from concourse import dve_ops
    from concourse.dve_ops import DveOp
    from concourse.dve_spec import Spec, Src0, C0, C1, C2

    name = "FRAC_CENTER"
    for o in dve_ops.OPS:
        if o.name == name:
            return o

    def make(sha):
        _a = Src0 * C0 + C1
        return DveOp(
            name,
            Spec(
                body=_a - ((_a + C2) - C2),
                reference=lambda in0, in1, c0, c1, c2: (
                    lambda a: (a - ((a + np.float32(c2)) - np.float32(c2))).astype(np.float32)
                )(np.float32(in0 * c0 + c1)),
            ),
            subdim=False,
            uops_sha={"v3": sha},
        )

    op = make(FRAC_SHA_V3)
    dve_ops.OPS.append(op)
    dve_ops._SUB_OPCODE_FOR_NAME[name] = dve_ops._CUSTOM_DVE_ROW_BASE + len(dve_ops.OPS) - 1
    dve_ops.CUSTOM_DVE_SPECS[name] = op.spec
    try:
        op.compile("v3")
    except ValueError as e:
        # sha drifted with toolchain; re-pin from the error message
        import re
        m = re.search(r"v3: (\w+)", str(e))
        dve_ops.OPS.remove(op)
        op = make(m.group(1))
        dve_ops.OPS.append(op)
        dve_ops.CUSTOM_DVE_SPECS[name] = op.spec
        op.compile("v3")
    return op


def _emb_perm(n_l):
    """Device emb row j -> reference feature row. Rows: 3*n_l sin (l-major),
    3*n_l cos, then x,y,z passthrough."""
    perm = []
    for rep in range(2 * n_l):
        l = rep % n_l
        t = 0 if rep < n_l else 1
        for ax in range(3):
            perm.append(3 + 6 * l + 3 * t + ax)
    perm += [0, 1, 2]
    return np.array(perm)


def _build(weights):
    """Build + compile the Bass program. weights: dict of prepped np arrays."""
    import concourse.bass as bass
    import concourse.tile as tile
    from concourse import bacc, mybir
    from contextlib import ExitStack

    F32 = mybir.dt.float32
    F32R = mybir.dt.float32r
    AF = mybir.ActivationFunctionType
    ALU = mybir.AluOpType

    frac_op = _build_frac_op()

    nc = bacc.Bacc("TRN2", target_bir_lowering=False, debug=False,
                   enable_asserts=False, num_devices=1)

    od_ap = nc.dram_tensor("od", [6, NPC], F32, kind="ExternalInput").ap()
    sv_ap = nc.dram_tensor("sv", [88, 2], F32, kind="ExternalInput").ap()
    bias_ap = nc.dram_tensor("bias", [128, 16], F32, kind="ExternalInput").ap()
    w_aps = {}
    for name in ("w10", "w11", "w12", "w13", "w20a", "w20b", "w21", "w22",
                 "w3e", "w3be", "wc"):
        w_aps[name] = nc.dram_tensor(name, list(weights[name].shape), F32R,
                                     kind="ExternalInput").ap()
    outc_ap = nc.dram_tensor("outc", [3, NPC], F32, kind="ExternalOutput").ap()
    outs_ap = nc.dram_tensor("outs", [1, NPC], F32R, kind="ExternalOutput").ap()

    # bias column indices
    BI = {"b10": 0, "b11": 1, "b12": 2, "b13": 3, "b20": 4, "b21": 5,
          "b22": 6, "b3e": 7, "bc": 8}

    with tile.TileContext(nc) as tc, ExitStack() as ctx:
        cpool = ctx.enter_context(tc.tile_pool(name="const", bufs=1))
        embp = ctx.enter_context(tc.tile_pool(name="emb", bufs=2))
        hp = ctx.enter_context(tc.tile_pool(name="h", bufs=8))
        h9p = ctx.enter_context(tc.tile_pool(name="h9", bufs=4))
        c3p = ctx.enter_context(tc.tile_pool(name="c3", bufs=4))
        psb = ctx.enter_context(tc.tile_pool(name="psb", bufs=4, space="PSUM"))

        wt = {}
        for name, ap in w_aps.items():
            wt[name] = cpool.tile(list(weights[name].shape), F32R, tag=name, name="wt_" + name)
            nc.sync.dma_start(wt[name][:], ap[:])
        svt = cpool.tile([88, 2], F32, tag="sv")
        nc.sync.dma_start(svt[:], sv_ap[:])
        bt = cpool.tile([128, 16], F32, tag="bias")
        nc.sync.dma_start(bt[:], bias_ap[:])

        def bias(name, rows=128):
            return bt[0:rows, BI[name]:BI[name] + 1]

        def relu_evac(eng, dst, src, bname, rows=128):
            if eng == "act":
                nc.scalar.activation(dst, src, AF.Relu, bias=bias(bname, rows))
            else:
                nc.vector.tensor_scalar(dst, src, bias(bname, rows), 0.0,
                                        ALU.add, ALU.max)

        n_batches = NPC // JB
        for b in range(n_batches):
            c0 = b * JB
            # ---- embedding stage (JB-wide) ----
            xb = embp.tile([88, JB], F32, tag="xb")
            src_pos = bass.AP(tensor=od_ap.tensor, offset=c0,
                              ap=[[0, 20], [NPC, 3], [1, JB]])
            nc.sync.dma_start(xb[0:60, :], src_pos)
            src_fill = bass.AP(tensor=od_ap.tensor, offset=c0,
                               ap=[[0, 4], [1, JB]])
            nc.sync.dma_start(xb[60:64, :], src_fill)
            src_dir = bass.AP(tensor=od_ap.tensor, offset=3 * NPC + c0,
                              ap=[[0, 8], [NPC, 3], [1, JB]])
            nc.sync.dma_start(xb[64:88, :], src_dir)

            f = embp.tile([88, JB], F32, tag="f")
            nc.vector._custom_dve(frac_op, out=f[:], in0=xb[:], in1=None,
                                  s0=svt[:, 0:1], s1=svt[:, 1:2], imm2=MAGIC)

            ex = embp.tile([63, JB], F32R, tag="ex")
            nc.scalar.activation(ex[0:60, :], f[0:60, :], AF.Sin, scale=TWO_PI)
            nc.gpsimd.dma_start(ex[60:63, :], od_ap[0:3, c0:c0 + JB])
            ed = embp.tile([27, JB], F32R, tag="ed")
            nc.scalar.activation(ed[0:24, :], f[64:88, :], AF.Sin, scale=TWO_PI)
            nc.gpsimd.dma_start(ed[24:27, :], od_ap[3:6, c0:c0 + JB])

            for j in range(JB // JS):
                jc = j * JS
                nsub = JS // TM

                def layer(wname, rhs_tile, rhs_rows, rhs_col0, bname, eng,
                          out_rows=128, extra=None):
                    """matmul (+optional K-split extra) then relu evac.
                    rhs_col0: column base of this subtile within rhs_tile."""
                    ps = psb.tile([out_rows, JS], mybir.dt.float32, tag="big", name="ps_" + wname)
                    for s in range(nsub):
                        rs = slice(rhs_col0 + s * TM, rhs_col0 + (s + 1) * TM)
                        nc.tensor.matmul(ps[:, s * TM:(s + 1) * TM],
                                         wt[wname][:], rhs_tile[0:rhs_rows, rs],
                                         start=True, stop=not extra)
                        if extra:
                            ew, et, er, ec0 = extra
                            es = slice(ec0 + s * TM, ec0 + (s + 1) * TM)
                            nc.tensor.matmul(ps[:, s * TM:(s + 1) * TM],
                                             wt[ew][:], et[0:er, es],
                                             start=False, stop=True)
                    h = (h9p if out_rows == 65 else hp).tile(
                        [out_rows, JS], F32R, tag="h9" if out_rows == 65 else "h",
                        name="h_" + wname)
                    relu_evac(eng, h[:], ps[:], bname, out_rows)
                    return h

                h1 = layer("w10", ex, 63, jc, "b10", "act")
                h2 = layer("w11", h1, 128, 0, "b11", "dve")
                h3 = layer("w12", h2, 128, 0, "b12", "act")
                h4 = layer("w13", h3, 128, 0, "b13", "dve")
                h5 = layer("w20a", h4, 128, 0, "b20", "dve",
                           extra=("w20b", ex, 63, jc))
                h6 = layer("w21", h5, 128, 0, "b21", "act")
                a22 = layer("w22", h6, 128, 0, "b22", "dve")
                h9e = layer("w3e", a22, 128, 0, "b3e", "dve", out_rows=65,
                            extra=("w3be", ed, 27, jc))

                # sigma: row 64 of h9e, already relu'd + biased
                nc.sync.dma_start(outs_ap[0:1, c0 + jc:c0 + jc + JS],
                                  h9e[64:65, :])

                # c logits: wc [64,3] @ h9e[0:64]
                psc = psb.tile([3, JS], mybir.dt.float32, tag="big", name="ps_c3")
                for s in range(nsub):
                    nc.tensor.matmul(psc[:, s * TM:(s + 1) * TM], wt["wc"][:],
                                     h9e[0:64, s * TM:(s + 1) * TM],
                                     start=True, stop=True)
                c3 = c3p.tile([3, JS], F32, tag="c3")
                nc.scalar.activation(c3[:], psc[:], AF.Identity, bias=bias("bc", 3))
                nc.sync.dma_start(outc_ap[:, c0 + jc:c0 + jc + JS], c3[:])

    nc.compile()
    return nc


def _prep(inputs):
    """Host-side weight folding and constant prep (float64 folds, f32 out)."""
    f64 = {k: np.asarray(v, np.float64) for k, v in inputs.items()
           if k not in ("o", "d")}
    perm_x = _emb_perm(L_POS)   # 63
    perm_d = _emb_perm(L_DIR)   # 27

    w10 = f64["w10"][perm_x]
    w20 = f64["w20"]
    w20a, w20b = w20[:128], w20[128:][perm_x]
    w23 = f64["w23"]
    w23c, w23s = w23[:, :128], w23[:, 128]
    b23 = f64["b23"]
    b23c, b23s = b23[:128], b23[128]
    w3 = f64["w3"]
    w3a, w3b = w3[:128], w3[128:][perm_d]

    w3p = w23c @ w3a                       # [128, 64]
    b3p = b23c @ w3a + f64["b3"]           # [64]
    w3e = np.concatenate([w3p, w23s[:, None]], axis=1)        # [128, 65]
    w3be = np.concatenate([w3b, np.zeros((27, 1))], axis=1)   # [27, 65]
    b3e = np.concatenate([b3p, [b23s]])                       # [65]

    W = dict(
        w10=w10, w11=f64["w11"], w12=f64["w12"], w13=f64["w13"],
        w20a=w20a, w20b=w20b, w21=f64["w21"], w22=f64["w22"],
        w3e=w3e, w3be=w3be, wc=f64["w4"],
    )
    # PE reads truncate fp32 operands to FP22 (toward zero): each w*x product
    # is low by ~2*2^-12 in expectation; compensate with a weight pre-scale.
    comp = 1.0 + float(os.environ.get("KERNEL_FP22_COMP", "0"))
    W = {k: np.ascontiguousarray(v * comp, dtype=np.float32) for k, v in W.items()}

    bias = np.zeros((128, 16), np.float32)
    for i, k in enumerate(("b10", "b11", "b12", "b13", "b20", "b21", "b22")):
        bias[:, i] = f64[k].astype(np.float32)
    bias[0:65, 7] = b3e.astype(np.float32)
    bias[0:3, 8] = f64["b4"].astype(np.float32)

    sv = np.zeros((88, 2), np.float32)
    for rep in range(20):
        l = rep % 10
        sv[rep * 3:rep * 3 + 3, 0] = (2.0 ** l) / TWO_PI
        sv[rep * 3:rep * 3 + 3, 1] = 0.0 if rep < 10 else 0.25
    for rep in range(8):
        l = rep % 4
        r0 = 64 + rep * 3
        sv[r0:r0 + 3, 0] = (2.0 ** l) / TWO_PI
        sv[r0:r0 + 3, 1] = 0.0 if rep < 4 else 0.25

    return W, bias, sv


def _run(inputs, trace=False, trace_kwargs=None):
    from concourse.bass_utils import run_bass_kernel_spmd

    W, bias, sv = _prep(inputs)
    key = "nc"
    if key not in _cache:
        _cache[key] = _build(W)
    nc = _cache[key]

    o = np.asarray(inputs["o"], np.float32)
    d = np.asarray(inputs["d"], np.float32)
    oT = np.ascontiguousarray(o.T)  # [3, N]
    dT = np.ascontiguousarray(d.T)

    in_maps = []
    for c in range(NC):
        cs, ce = c * NPC, (c + 1) * NPC
        od = np.ascontiguousarray(np.concatenate([oT[:, cs:ce], dT[:, cs:ce]], axis=0))
        m = dict(od=od, sv=sv, bias=bias)
        for k, v in W.items():
            m[k] = v
        in_maps.append(m)

    kw = {}
    if trace:
        kw["trace"] = True
        if trace_kwargs:
            kw.update(trace_kwargs)
    res = run_bass_kernel_spmd(nc, in_maps, core_ids=list(range(NC)), **kw)

    c_parts, s_parts = [], []
    for cr in range(NC):
        logits = res.results[cr]["outc"]           # [3, NPC]
        sig = res.results[cr]["outs"][0]           # [NPC]
        c_parts.append(0.5 * (1.0 + np.tanh(0.5 * logits.astype(np.float64))))
        s_parts.append(sig)
    c = np.ascontiguousarray(np.concatenate(c_parts, axis=1).T.astype(np.float32))
    sigma = np.concatenate(s_parts).astype(np.float32)
    return (c, sigma), res


def kernel(**inputs):
    (c, sigma), _ = _run(inputs)
    return c, sigma


# revision 10
# speedup vs baseline: 1.8843x; 1.6252x over previous
"""NeRF MLP forward on 8 Trainium2 NeuronCores (Bass/Tile), data-parallel.

Layout: feature-major ([features, points]) so the MLP chains through the
PE array with weights as the stationary operand (lhsT = W[fan_in, fan_out]).

Positional encoding: args = x * (2^l / 2pi) + (0 | 0.25) are range-reduced
with a custom DVE op (frac-center via the +-1.5*2^23 magic-number trick) in a
single instruction, then evaluated with one ACT Sin op per block
(sin(2pi f) covers both sin and cos rows; the quarter-turn shift is applied
before reduction so the argument stays within the LUT domain [-pi, pi]).

Algebraic folds (host-side, exact):
  - t = a22 @ w23 + b23 has no activation on its first 128 cols, so
    W3' = w23c @ w3a and b3' = b23c @ w3a + b3 fold layer 23c away.
  - sigma shares rhs a22 with W3', so it rides as column 64 of an M=65
    matmul; one relu evac produces h9 and final sigma together.
  - final sigmoid is applied on host to the DMA'd logits (the Sin table set
    has no sigmoid and a per-tile ACT table switch costs ~2.7us).
"""
import os
import sys
import numpy as np

NC = 8
N_TOTAL = 524288
NPC = N_TOTAL // NC  # 65536 points per core
HID = 128
L_POS = 10
L_DIR = 4
JB = 2048    # emb batch (bcast/frac/sin granularity)
JS = 1024    # evac sub-tile granularity
TM = 512     # matmul free-dim tile
TWO_PI = 2.0 * np.pi
MAGIC = 12582912.0  # 1.5 * 2**23  (fp32 round-to-nearest-int trick)

FRAC_SHA_V3 = "3d790cc1ec454799"

_cache = {}


def _build_frac_op():
    """out = a - round(a), a = Src0*C0 + C1  (one DVE pass)."""
    from concourse import dve_ops
    from concourse.dve_ops import DveOp
    from concourse.dve_spec import Spec, Src0, C0, C1, C2

    name = "FRAC_CENTER"
    for o in dve_ops.OPS:
        if o.name == name:
            return o

    def make(sha):
        _a = Src0 * C0 + C1
        return DveOp(
            name,
            Spec(
                body=_a - ((_a + C2) - C2),
                reference=lambda in0, in1, c0, c1, c2: (
                    lambda a: (a - ((a + np.float32(c2)) - np.float32(c2))).astype(np.float32)
                )(np.float32(in0 * c0 + c1)),
            ),
            subdim=False,
            uops_sha={"v3": sha},
        )

    op = make(FRAC_SHA_V3)
    dve_ops.OPS.append(op)
    dve_ops._SUB_OPCODE_FOR_NAME[name] = dve_ops._CUSTOM_DVE_ROW_BASE + len(dve_ops.OPS) - 1
    dve_ops.CUSTOM_DVE_SPECS[name] = op.spec
    try:
        op.compile("v3")
    except ValueError as e:
        # sha drifted with toolchain; re-pin from the error message
        import re
        m = re.search(r"v3: (\w+)", str(e))
        dve_ops.OPS.remove(op)
        op = make(m.group(1))
        dve_ops.OPS.append(op)
        dve_ops.CUSTOM_DVE_SPECS[name] = op.spec
        op.compile("v3")
    return op


def _emb_perm(n_l):
    """Device emb row j -> reference feature row. Rows: 3*n_l sin (l-major),
    3*n_l cos, then x,y,z passthrough."""
    perm = []
    for rep in range(2 * n_l):
        l = rep % n_l
        t = 0 if rep < n_l else 1
        for ax in range(3):
            perm.append(3 + 6 * l + 3 * t + ax)
    perm += [0, 1, 2]
    return np.array(perm)


def _build(weights):
    """Build + compile the Bass program. weights: dict of prepped np arrays."""
    import concourse.bass as bass
    import concourse.tile as tile
    from concourse import bacc, mybir
    from contextlib import ExitStack

    F32 = mybir.dt.float32
    F32R = mybir.dt.float32r
    AF = mybir.ActivationFunctionType
    ALU = mybir.AluOpType

    frac_op = _build_frac_op()

    nc = bacc.Bacc("TRN2", target_bir_lowering=False, debug=False,
                   enable_asserts=False, num_devices=1)

    od_ap = nc.dram_tensor("od", [6, NPC], F32, kind="ExternalInput").ap()
    sv_ap = nc.dram_tensor("sv", [88, 2], F32, kind="ExternalInput").ap()
    bias_ap = nc.dram_tensor("bias", [128, 16], F32, kind="ExternalInput").ap()
    w_aps = {}
    for name in ("w10", "w11", "w12", "w13", "w20a", "w20b", "w21", "w22",
                 "w3e", "w3be", "wc"):
        w_aps[name] = nc.dram_tensor(name, list(weights[name].shape), F32R,
                                     kind="ExternalInput").ap()
    outc_ap = nc.dram_tensor("outc", [3, NPC], F32, kind="ExternalOutput").ap()
    outs_ap = nc.dram_tensor("outs", [1, NPC], F32R, kind="ExternalOutput").ap()

    # bias column indices
    BI = {"b10": 0, "b11": 1, "b12": 2, "b13": 3, "b20": 4, "b21": 5,
          "b22": 6, "b3e": 7, "bc": 8}

    with tile.TileContext(nc) as tc, ExitStack() as ctx:
        cpool = ctx.enter_context(tc.tile_pool(name="const", bufs=1))
        embp = ctx.enter_context(tc.tile_pool(name="emb", bufs=2))
        hp = ctx.enter_context(tc.tile_pool(name="h", bufs=8))
        h9p = ctx.enter_context(tc.tile_pool(name="h9", bufs=4))
        c3p = ctx.enter_context(tc.tile_pool(name="c3", bufs=4))
        psb = ctx.enter_context(tc.tile_pool(name="psb", bufs=4, space="PSUM"))

        wt = {}
        for name, ap in w_aps.items():
            wt[name] = cpool.tile(list(weights[name].shape), F32R, tag=name, name="wt_" + name)
            nc.sync.dma_start(wt[name][:], ap[:])
        svt = cpool.tile([88, 2], F32, tag="sv")
        nc.sync.dma_start(svt[:], sv_ap[:])
        bt = cpool.tile([128, 16], F32, tag="bias")
        nc.sync.dma_start(bt[:], bias_ap[:])

        def bias(name, rows=128):
            return bt[0:rows, BI[name]:BI[name] + 1]

        def relu_evac(eng, dst, src, bname, rows=128):
            if eng == "act":
                nc.scalar.activation(dst, src, AF.Relu, bias=bias(bname, rows))
            else:
                nc.vector.tensor_scalar(dst, src, bias(bname, rows), 0.0,
                                        ALU.add, ALU.max)

        n_batches = NPC // JB
        for b in range(n_batches):
            c0 = b * JB
            # ---- embedding stage (JB-wide) ----
            xb = embp.tile([88, JB], F32, tag="xb")
            src_pos = bass.AP(tensor=od_ap.tensor, offset=c0,
                              ap=[[0, 20], [NPC, 3], [1, JB]])
            nc.sync.dma_start(xb[0:60, :], src_pos)
            src_fill = bass.AP(tensor=od_ap.tensor, offset=c0,
                               ap=[[0, 4], [1, JB]])
            nc.sync.dma_start(xb[60:64, :], src_fill)
            src_dir = bass.AP(tensor=od_ap.tensor, offset=3 * NPC + c0,
                              ap=[[0, 8], [NPC, 3], [1, JB]])
            nc.sync.dma_start(xb[64:88, :], src_dir)

            f = embp.tile([88, JB], F32, tag="f")
            nc.vector._custom_dve(frac_op, out=f[:], in0=xb[:], in1=None,
                                  s0=svt[:, 0:1], s1=svt[:, 1:2], imm2=MAGIC)

            ex = embp.tile([63, JB], F32R, tag="ex")
            nc.scalar.activation(ex[0:60, :], f[0:60, :], AF.Sin, scale=TWO_PI)
            nc.gpsimd.dma_start(ex[60:63, :], od_ap[0:3, c0:c0 + JB])
            ed = embp.tile([27, JB], F32R, tag="ed")
            nc.scalar.activation(ed[0:24, :], f[64:88, :], AF.Sin, scale=TWO_PI)
            nc.gpsimd.dma_start(ed[24:27, :], od_ap[3:6, c0:c0 + JB])

            nsub = JS // TM
            nchain = JB // JS  # lockstep chains per emb batch

            def mm_layer(wname, ps, rhs_tile, rhs_rows, rhs_col0, extra=None):
                for s in range(nsub):
                    rs = slice(rhs_col0 + s * TM, rhs_col0 + (s + 1) * TM)
                    nc.tensor.matmul(ps[:, s * TM:(s + 1) * TM],
                                     wt[wname][:], rhs_tile[0:rhs_rows, rs],
                                     start=True, stop=not extra)
                    if extra:
                        ew, et, er, ec0 = extra
                        es = slice(ec0 + s * TM, ec0 + (s + 1) * TM)
                        nc.tensor.matmul(ps[:, s * TM:(s + 1) * TM],
                                         wt[ew][:], et[0:er, es],
                                         start=False, stop=True)

            # lockstep chains: layer-by-layer across nchain subtiles; evacs
            # checkerboard across ACT/DVE so same-layer evacs run in parallel
            LAYERS = [
                ("w10", "b10", "ex", None),
                ("w11", "b11", "h", None),
                ("w12", "b12", "h", None),
                ("w13", "b13", "h", None),
                ("w20a", "b20", "h", ("w20b", "ex", 63)),
                ("w21", "b21", "h", None),
                ("w22", "b22", "h", None),
                ("w3e", "b3e", "h", ("w3be", "ed", 27)),
            ]
            hcur = [None] * nchain
            for li, (wname, bname, rhs_kind, extra) in enumerate(LAYERS):
                out_rows = 65 if wname == "w3e" else 128
                for j in range(nchain):
                    jc = j * JS
                    ps = psb.tile([out_rows, JS], mybir.dt.float32, tag="big",
                                  name=f"ps_{wname}_{j}")
                    if rhs_kind == "ex":
                        mm_layer(wname, ps, ex, 63, jc)
                    else:
                        ex_extra = None
                        if extra:
                            ew, ekind, er = extra
                            et = ex if ekind == "ex" else ed
                            ex_extra = (ew, et, er, jc)
                        mm_layer(wname, ps, hcur[j], 128, 0, extra=ex_extra)
                    h = (h9p if out_rows == 65 else hp).tile(
                        [out_rows, JS], F32R,
                        tag="h9" if out_rows == 65 else "h",
                        name=f"h_{wname}_{j}")
                    eng = "act" if (li + j) % 2 == 0 else "dve"
                    relu_evac(eng, h[:], ps[:], bname, out_rows)
                    hcur[j] = h

            for j in range(nchain):
                jc = j * JS
                h9e = hcur[j]
                nc.sync.dma_start(outs_ap[0:1, c0 + jc:c0 + jc + JS],
                                  h9e[64:65, :])
                psc = psb.tile([3, JS], mybir.dt.float32, tag="big",
                               name=f"ps_c3_{j}")
                for s in range(nsub):
                    nc.tensor.matmul(psc[:, s * TM:(s + 1) * TM], wt["wc"][:],
                                     h9e[0:64, s * TM:(s + 1) * TM],
                                     start=True, stop=True)
                c3 = c3p.tile([3, JS], F32, tag="c3", name=f"c3_{j}")
                eng = "act" if j % 2 == 0 else "dve"
                if eng == "act":
                    nc.scalar.activation(c3[:], psc[:], AF.Identity, bias=bias("bc", 3))
                else:
                    nc.vector.tensor_scalar(c3[:], psc[:], bias("bc", 3), None, ALU.add)
                nc.sync.dma_start(outc_ap[:, c0 + jc:c0 + jc + JS], c3[:])

    nc.compile()
    return nc


def _prep(inputs):
    """Host-side weight folding and constant prep (float64 folds, f32 out)."""
    f64 = {k: np.asarray(v, np.float64) for k, v in inputs.items()
           if k not in ("o", "d")}
    perm_x = _emb_perm(L_POS)   # 63
    perm_d = _emb_perm(L_DIR)   # 27

    w10 = f64["w10"][perm_x]
    w20 = f64["w20"]
    w20a, w20b = w20[:128], w20[128:][perm_x]
    w23 = f64["w23"]
    w23c, w23s = w23[:, :128], w23[:, 128]
    b23 = f64["b23"]
    b23c, b23s = b23[:128], b23[128]
    w3 = f64["w3"]
    w3a, w3b = w3[:128], w3[128:][perm_d]

    w3p = w23c @ w3a                       # [128, 64]
    b3p = b23c @ w3a + f64["b3"]           # [64]
    w3e = np.concatenate([w3p, w23s[:, None]], axis=1)        # [128, 65]
    w3be = np.concatenate([w3b, np.zeros((27, 1))], axis=1)   # [27, 65]
    b3e = np.concatenate([b3p, [b23s]])                       # [65]

    W = dict(
        w10=w10, w11=f64["w11"], w12=f64["w12"], w13=f64["w13"],
        w20a=w20a, w20b=w20b, w21=f64["w21"], w22=f64["w22"],
        w3e=w3e, w3be=w3be, wc=f64["w4"],
    )
    # PE reads truncate fp32 operands to FP22 (toward zero): each w*x product
    # is low by ~2*2^-12 in expectation; compensate with a weight pre-scale.
    comp = 1.0 + float(os.environ.get("KERNEL_FP22_COMP", "0"))
    W = {k: np.ascontiguousarray(v * comp, dtype=np.float32) for k, v in W.items()}

    bias = np.zeros((128, 16), np.float32)
    for i, k in enumerate(("b10", "b11", "b12", "b13", "b20", "b21", "b22")):
        bias[:, i] = f64[k].astype(np.float32)
    bias[0:65, 7] = b3e.astype(np.float32)
    bias[0:3, 8] = f64["b4"].astype(np.float32)

    sv = np.zeros((88, 2), np.float32)
    for rep in range(20):
        l = rep % 10
        sv[rep * 3:rep * 3 + 3, 0] = (2.0 ** l) / TWO_PI
        sv[rep * 3:rep * 3 + 3, 1] = 0.0 if rep < 10 else 0.25
    for rep in range(8):
        l = rep % 4
        r0 = 64 + rep * 3
        sv[r0:r0 + 3, 0] = (2.0 ** l) / TWO_PI
        sv[r0:r0 + 3, 1] = 0.0 if rep < 4 else 0.25

    return W, bias, sv


def _run(inputs, trace=False, trace_kwargs=None):
    from concourse.bass_utils import run_bass_kernel_spmd

    W, bias, sv = _prep(inputs)
    key = "nc"
    if key not in _cache:
        _cache[key] = _build(W)
    nc = _cache[key]

    o = np.asarray(inputs["o"], np.float32)
    d = np.asarray(inputs["d"], np.float32)
    oT = np.ascontiguousarray(o.T)  # [3, N]
    dT = np.ascontiguousarray(d.T)

    in_maps = []
    for c in range(NC):
        cs, ce = c * NPC, (c + 1) * NPC
        od = np.ascontiguousarray(np.concatenate([oT[:, cs:ce], dT[:, cs:ce]], axis=0))
        m = dict(od=od, sv=sv, bias=bias)
        for k, v in W.items():
            m[k] = v
        in_maps.append(m)

    kw = {}
    if trace:
        kw["trace"] = True
        if trace_kwargs:
            kw.update(trace_kwargs)
    res = run_bass_kernel_spmd(nc, in_maps, core_ids=list(range(NC)), **kw)

    c_parts, s_parts = [], []
    for cr in range(NC):
        logits = res.results[cr]["outc"]           # [3, NPC]
        sig = res.results[cr]["outs"][0]           # [NPC]
        c_parts.append(0.5 * (1.0 + np.tanh(0.5 * logits.astype(np.float64))))
        s_parts.append(sig)
    c = np.ascontiguousarray(np.concatenate(c_parts, axis=1).T.astype(np.float32))
    sigma = np.concatenate(s_parts).astype(np.float32)
    return (c, sigma), res


def kernel(**inputs):
    (c, sigma), _ = _run(inputs)
    return c, sigma


# revision 11
# speedup vs baseline: 2.6025x; 1.3811x over previous
"""NeRF MLP forward on 8 Trainium2 NeuronCores (Bass/Tile), data-parallel.

Layout: feature-major ([features, points]) so the MLP chains through the
PE array with weights as the stationary operand (lhsT = W[fan_in, fan_out]).

Positional encoding: args = x * (2^l / 2pi) + (0 | 0.25) are range-reduced
with a custom DVE op (frac-center via the +-1.5*2^23 magic-number trick) in a
single instruction, then evaluated with one ACT Sin op per block
(sin(2pi f) covers both sin and cos rows; the quarter-turn shift is applied
before reduction so the argument stays within the LUT domain [-pi, pi]).

Algebraic folds (host-side, exact):
  - t = a22 @ w23 + b23 has no activation on its first 128 cols, so
    W3' = w23c @ w3a and b3' = b23c @ w3a + b3 fold layer 23c away.
  - sigma shares rhs a22 with W3', so it rides as column 64 of an M=65
    matmul; one relu evac produces h9 and final sigma together.
  - final sigmoid is applied on host to the DMA'd logits (the Sin table set
    has no sigmoid and a per-tile ACT table switch costs ~2.7us).
"""
import os
import sys
import numpy as np

NC = 8
N_TOTAL = 524288
NPC = N_TOTAL // NC  # 65536 points per core
HID = 128
L_POS = 10
L_DIR = 4
JB = 4096    # emb batch (bcast/frac/sin granularity)
JS = 1024    # evac sub-tile granularity
TM = 512     # matmul free-dim tile
TWO_PI = 2.0 * np.pi
MAGIC = 12582912.0  # 1.5 * 2**23  (fp32 round-to-nearest-int trick)

FRAC_SHA_V3 = "3d790cc1ec454799"

_cache = {}


def _build_frac_op():
    """out = a - round(a), a = Src0*C0 + C1  (one DVE pass)."""
    from concourse import dve_ops
    from concourse.dve_ops import DveOp
    from concourse.dve_spec import Spec, Src0, C0, C1, C2

    name = "FRAC_CENTER"
    for o in dve_ops.OPS:
        if o.name == name:
            return o

    def make(sha):
        _a = Src0 * C0 + C1
        return DveOp(
            name,
            Spec(
                body=_a - ((_a + C2) - C2),
                reference=lambda in0, in1, c0, c1, c2: (
                    lambda a: (a - ((a + np.float32(c2)) - np.float32(c2))).astype(np.float32)
                )(np.float32(in0 * c0 + c1)),
            ),
            subdim=False,
            uops_sha={"v3": sha},
        )

    op = make(FRAC_SHA_V3)
    dve_ops.OPS.append(op)
    dve_ops._SUB_OPCODE_FOR_NAME[name] = dve_ops._CUSTOM_DVE_ROW_BASE + len(dve_ops.OPS) - 1
    dve_ops.CUSTOM_DVE_SPECS[name] = op.spec
    try:
        op.compile("v3")
    except ValueError as e:
        # sha drifted with toolchain; re-pin from the error message
        import re
        m = re.search(r"v3: (\w+)", str(e))
        dve_ops.OPS.remove(op)
        op = make(m.group(1))
        dve_ops.OPS.append(op)
        dve_ops.CUSTOM_DVE_SPECS[name] = op.spec
        op.compile("v3")
    return op


def _emb_perm(n_l):
    """Device emb row j -> reference feature row. Rows: 3*n_l sin (l-major),
    3*n_l cos, then x,y,z passthrough."""
    perm = []
    for rep in range(2 * n_l):
        l = rep % n_l
        t = 0 if rep < n_l else 1
        for ax in range(3):
            perm.append(3 + 6 * l + 3 * t + ax)
    perm += [0, 1, 2]
    return np.array(perm)


def _build(weights):
    """Build + compile the Bass program. weights: dict of prepped np arrays."""
    import concourse.bass as bass
    import concourse.tile as tile
    from concourse import bacc, mybir
    from contextlib import ExitStack

    F32 = mybir.dt.float32
    F32R = mybir.dt.float32r
    AF = mybir.ActivationFunctionType
    ALU = mybir.AluOpType

    frac_op = _build_frac_op()

    nc = bacc.Bacc("TRN2", target_bir_lowering=False, debug=False,
                   enable_asserts=False, num_devices=1)

    od_ap = nc.dram_tensor("od", [6, NPC], F32, kind="ExternalInput").ap()
    sv_ap = nc.dram_tensor("sv", [88, 2], F32, kind="ExternalInput").ap()
    bias_ap = nc.dram_tensor("bias", [128, 16], F32, kind="ExternalInput").ap()
    w_aps = {}
    for name in ("w10", "w11", "w12", "w13", "w20a", "w20b", "w21", "w22",
                 "w3e", "w3be", "wc"):
        w_aps[name] = nc.dram_tensor(name, list(weights[name].shape), F32R,
                                     kind="ExternalInput").ap()
    outc_ap = nc.dram_tensor("outc", [3, NPC], F32, kind="ExternalOutput").ap()
    outs_ap = nc.dram_tensor("outs", [1, NPC], F32R, kind="ExternalOutput").ap()

    # bias column indices
    BI = {"b10": 0, "b11": 1, "b12": 2, "b13": 3, "b20": 4, "b21": 5,
          "b22": 6, "b3e": 7, "bc": 8}

    with tile.TileContext(nc) as tc, ExitStack() as ctx:
        cpool = ctx.enter_context(tc.tile_pool(name="const", bufs=1))
        embp = ctx.enter_context(tc.tile_pool(name="emb", bufs=2))
        hp = ctx.enter_context(tc.tile_pool(name="h", bufs=6))
        h9p = ctx.enter_context(tc.tile_pool(name="h9", bufs=3))
        c3p = ctx.enter_context(tc.tile_pool(name="c3", bufs=3))
        psb = ctx.enter_context(tc.tile_pool(name="psb", bufs=4, space="PSUM"))

        wt = {}
        for name, ap in w_aps.items():
            wt[name] = cpool.tile(list(weights[name].shape), F32R, tag=name, name="wt_" + name)
            nc.sync.dma_start(wt[name][:], ap[:])
        svt = cpool.tile([88, 2], F32, tag="sv")
        nc.sync.dma_start(svt[:], sv_ap[:])
        bt = cpool.tile([128, 16], F32, tag="bias")
        nc.sync.dma_start(bt[:], bias_ap[:])

        def bias(name, rows=128):
            return bt[0:rows, BI[name]:BI[name] + 1]

        def relu_evac(eng, dst, src, bname, rows=128):
            if eng == "act":
                nc.scalar.activation(dst, src, AF.Relu, bias=bias(bname, rows))
            else:
                nc.vector.tensor_scalar(dst, src, bias(bname, rows), 0.0,
                                        ALU.add, ALU.max)

        n_batches = NPC // JB
        for b in range(n_batches):
            c0 = b * JB
            # ---- embedding stage (JB-wide) ----
            xb = embp.tile([88, JB], F32, tag="xb")
            src_pos = bass.AP(tensor=od_ap.tensor, offset=c0,
                              ap=[[0, 20], [NPC, 3], [1, JB]])
            nc.sync.dma_start(xb[0:60, :], src_pos)
            src_fill = bass.AP(tensor=od_ap.tensor, offset=c0,
                               ap=[[0, 4], [1, JB]])
            nc.sync.dma_start(xb[60:64, :], src_fill)
            src_dir = bass.AP(tensor=od_ap.tensor, offset=3 * NPC + c0,
                              ap=[[0, 8], [NPC, 3], [1, JB]])
            nc.sync.dma_start(xb[64:88, :], src_dir)

            f = embp.tile([88, JB], F32, tag="f")
            nc.vector._custom_dve(frac_op, out=f[:], in0=xb[:], in1=None,
                                  s0=svt[:, 0:1], s1=svt[:, 1:2], imm2=MAGIC)

            ex = embp.tile([63, JB], F32R, tag="ex")
            nc.scalar.activation(ex[0:60, :], f[0:60, :], AF.Sin, scale=TWO_PI)
            nc.gpsimd.dma_start(ex[60:63, :], od_ap[0:3, c0:c0 + JB])
            ed = embp.tile([27, JB], F32R, tag="ed")
            nc.scalar.activation(ed[0:24, :], f[64:88, :], AF.Sin, scale=TWO_PI)
            nc.gpsimd.dma_start(ed[24:27, :], od_ap[3:6, c0:c0 + JB])

            nsub = JS // TM
            nchain = JB // JS  # lockstep chains per emb batch

            def mm_layer(wname, ps, rhs_tile, rhs_rows, rhs_col0, extra=None):
                for s in range(nsub):
                    rs = slice(rhs_col0 + s * TM, rhs_col0 + (s + 1) * TM)
                    nc.tensor.matmul(ps[:, s * TM:(s + 1) * TM],
                                     wt[wname][:], rhs_tile[0:rhs_rows, rs],
                                     start=True, stop=not extra)
                    if extra:
                        ew, et, er, ec0 = extra
                        es = slice(ec0 + s * TM, ec0 + (s + 1) * TM)
                        nc.tensor.matmul(ps[:, s * TM:(s + 1) * TM],
                                         wt[ew][:], et[0:er, es],
                                         start=False, stop=True)

            # lockstep chains: layer-by-layer across nchain subtiles; evacs
            # checkerboard across ACT/DVE so same-layer evacs run in parallel
            LAYERS = [
                ("w10", "b10", "ex", None),
                ("w11", "b11", "h", None),
                ("w12", "b12", "h", None),
                ("w13", "b13", "h", None),
                ("w20a", "b20", "h", ("w20b", "ex", 63)),
                ("w21", "b21", "h", None),
                ("w22", "b22", "h", None),
                ("w3e", "b3e", "h", ("w3be", "ed", 27)),
            ]
            hcur = [None] * nchain
            for li, (wname, bname, rhs_kind, extra) in enumerate(LAYERS):
                out_rows = 65 if wname == "w3e" else 128
                for j in range(nchain):
                    jc = j * JS
                    ps = psb.tile([out_rows, JS], mybir.dt.float32, tag="big",
                                  name=f"ps_{wname}_{j}")
                    if rhs_kind == "ex":
                        mm_layer(wname, ps, ex, 63, jc)
                    else:
                        ex_extra = None
                        if extra:
                            ew, ekind, er = extra
                            et = ex if ekind == "ex" else ed
                            ex_extra = (ew, et, er, jc)
                        mm_layer(wname, ps, hcur[j], 128, 0, extra=ex_extra)
                    h = (h9p if out_rows == 65 else hp).tile(
                        [out_rows, JS], F32R,
                        tag="h9" if out_rows == 65 else "h",
                        name=f"h_{wname}_{j}")
                    eng = "act" if (li + j) % 2 == 0 else "dve"
                    relu_evac(eng, h[:], ps[:], bname, out_rows)
                    hcur[j] = h

            for j in range(nchain):
                jc = j * JS
                h9e = hcur[j]
                nc.sync.dma_start(outs_ap[0:1, c0 + jc:c0 + jc + JS],
                                  h9e[64:65, :])
                psc = psb.tile([3, JS], mybir.dt.float32, tag="big",
                               name=f"ps_c3_{j}")
                for s in range(nsub):
                    nc.tensor.matmul(psc[:, s * TM:(s + 1) * TM], wt["wc"][:],
                                     h9e[0:64, s * TM:(s + 1) * TM],
                                     start=True, stop=True)
                c3 = c3p.tile([3, JS], F32, tag="c3", name=f"c3_{j}")
                eng = "act" if j % 2 == 0 else "dve"
                if eng == "act":
                    nc.scalar.activation(c3[:], psc[:], AF.Identity, bias=bias("bc", 3))
                else:
                    nc.vector.tensor_scalar(c3[:], psc[:], bias("bc", 3), None, ALU.add)
                nc.sync.dma_start(outc_ap[:, c0 + jc:c0 + jc + JS], c3[:])

    nc.compile()
    return nc


def _prep(inputs):
    """Host-side weight folding and constant prep (float64 folds, f32 out)."""
    f64 = {k: np.asarray(v, np.float64) for k, v in inputs.items()
           if k not in ("o", "d")}
    perm_x = _emb_perm(L_POS)   # 63
    perm_d = _emb_perm(L_DIR)   # 27

    w10 = f64["w10"][perm_x]
    w20 = f64["w20"]
    w20a, w20b = w20[:128], w20[128:][perm_x]
    w23 = f64["w23"]
    w23c, w23s = w23[:, :128], w23[:, 128]
    b23 = f64["b23"]
    b23c, b23s = b23[:128], b23[128]
    w3 = f64["w3"]
    w3a, w3b = w3[:128], w3[128:][perm_d]

    w3p = w23c @ w3a                       # [128, 64]
    b3p = b23c @ w3a + f64["b3"]           # [64]
    w3e = np.concatenate([w3p, w23s[:, None]], axis=1)        # [128, 65]
    w3be = np.concatenate([w3b, np.zeros((27, 1))], axis=1)   # [27, 65]
    b3e = np.concatenate([b3p, [b23s]])                       # [65]

    W = dict(
        w10=w10, w11=f64["w11"], w12=f64["w12"], w13=f64["w13"],
        w20a=w20a, w20b=w20b, w21=f64["w21"], w22=f64["w22"],
        w3e=w3e, w3be=w3be, wc=f64["w4"],
    )
    # PE reads truncate fp32 operands to FP22 (toward zero): each w*x product
    # is low by ~2*2^-12 in expectation; compensate with a weight pre-scale.
    comp = 1.0 + float(os.environ.get("KERNEL_FP22_COMP", "0"))
    W = {k: np.ascontiguousarray(v * comp, dtype=np.float32) for k, v in W.items()}

    bias = np.zeros((128, 16), np.float32)
    for i, k in enumerate(("b10", "b11", "b12", "b13", "b20", "b21", "b22")):
        bias[:, i] = f64[k].astype(np.float32)
    bias[0:65, 7] = b3e.astype(np.float32)
    bias[0:3, 8] = f64["b4"].astype(np.float32)

    sv = np.zeros((88, 2), np.float32)
    for rep in range(20):
        l = rep % 10
        sv[rep * 3:rep * 3 + 3, 0] = (2.0 ** l) / TWO_PI
        sv[rep * 3:rep * 3 + 3, 1] = 0.0 if rep < 10 else 0.25
    for rep in range(8):
        l = rep % 4
        r0 = 64 + rep * 3
        sv[r0:r0 + 3, 0] = (2.0 ** l) / TWO_PI
        sv[r0:r0 + 3, 1] = 0.0 if rep < 4 else 0.25

    return W, bias, sv


def _run(inputs, trace=False, trace_kwargs=None):
    from concourse.bass_utils import run_bass_kernel_spmd

    W, bias, sv = _prep(inputs)
    key = "nc"
    if key not in _cache:
        _cache[key] = _build(W)
    nc = _cache[key]

    o = np.asarray(inputs["o"], np.float32)
    d = np.asarray(inputs["d"], np.float32)
    oT = np.ascontiguousarray(o.T)  # [3, N]
    dT = np.ascontiguousarray(d.T)

    in_maps = []
    for c in range(NC):
        cs, ce = c * NPC, (c + 1) * NPC
        od = np.ascontiguousarray(np.concatenate([oT[:, cs:ce], dT[:, cs:ce]], axis=0))
        m = dict(od=od, sv=sv, bias=bias)
        for k, v in W.items():
            m[k] = v
        in_maps.append(m)

    kw = {}
    if trace:
        kw["trace"] = True
        if trace_kwargs:
            kw.update(trace_kwargs)
    res = run_bass_kernel_spmd(nc, in_maps, core_ids=list(range(NC)), **kw)

    c_parts, s_parts = [], []
    for cr in range(NC):
        logits = res.results[cr]["outc"]           # [3, NPC]
        sig = res.results[cr]["outs"][0]           # [NPC]
        c_parts.append(0.5 * (1.0 + np.tanh(0.5 * logits.astype(np.float64))))
        s_parts.append(sig)
    c = np.ascontiguousarray(np.concatenate(c_parts, axis=1).T.astype(np.float32))
    sigma = np.concatenate(s_parts).astype(np.float32)
    return (c, sigma), res


def kernel(**inputs):
    (c, sigma), _ = _run(inputs)
    return c, sigma


# revision 12
# speedup vs baseline: 2.6720x; 1.0267x over previous
"""NeRF MLP forward on 8 Trainium2 NeuronCores (Bass/Tile), data-parallel.

Layout: feature-major ([features, points]) so the MLP chains through the
PE array with weights as the stationary operand (lhsT = W[fan_in, fan_out]).

Positional encoding: args = x * (2^l / 2pi) + (0 | 0.25) are range-reduced
with a custom DVE op (frac-center via the +-1.5*2^23 magic-number trick) in a
single instruction, then evaluated with one ACT Sin op per block
(sin(2pi f) covers both sin and cos rows; the quarter-turn shift is applied
before reduction so the argument stays within the LUT domain [-pi, pi]).

Algebraic folds (host-side, exact):
  - t = a22 @ w23 + b23 has no activation on its first 128 cols, so
    W3' = w23c @ w3a and b3' = b23c @ w3a + b3 fold layer 23c away.
  - sigma shares rhs a22 with W3', so it rides as column 64 of an M=65
    matmul; one relu evac produces h9 and final sigma together.
  - final sigmoid is applied on host to the DMA'd logits (the Sin table set
    has no sigmoid and a per-tile ACT table switch costs ~2.7us).
"""
import os
import sys
import numpy as np

NC = 8
N_TOTAL = 524288
NPC = N_TOTAL // NC  # 65536 points per core
HID = 128
L_POS = 10
L_DIR = 4
JB = 4096    # emb batch (bcast/frac/sin granularity)
JS = 1024    # evac sub-tile granularity
TM = 512     # matmul free-dim tile
TWO_PI = 2.0 * np.pi
MAGIC = 12582912.0  # 1.5 * 2**23  (fp32 round-to-nearest-int trick)

FRAC_SHA_V3 = "3d790cc1ec454799"

_cache = {}


def _build_frac_op():
    """out = a - round(a), a = Src0*C0 + C1  (one DVE pass)."""
    from concourse import dve_ops
    from concourse.dve_ops import DveOp
    from concourse.dve_spec import Spec, Src0, C0, C1, C2

    name = "FRAC_CENTER"
    for o in dve_ops.OPS:
        if o.name == name:
            return o

    def make(sha):
        _a = Src0 * C0 + C1
        return DveOp(
            name,
            Spec(
                body=_a - ((_a + C2) - C2),
                reference=lambda in0, in1, c0, c1, c2: (
                    lambda a: (a - ((a + np.float32(c2)) - np.float32(c2))).astype(np.float32)
                )(np.float32(in0 * c0 + c1)),
            ),
            subdim=False,
            uops_sha={"v3": sha},
        )

    op = make(FRAC_SHA_V3)
    dve_ops.OPS.append(op)
    dve_ops._SUB_OPCODE_FOR_NAME[name] = dve_ops._CUSTOM_DVE_ROW_BASE + len(dve_ops.OPS) - 1
    dve_ops.CUSTOM_DVE_SPECS[name] = op.spec
    try:
        op.compile("v3")
    except ValueError as e:
        # sha drifted with toolchain; re-pin from the error message
        import re
        m = re.search(r"v3: (\w+)", str(e))
        dve_ops.OPS.remove(op)
        op = make(m.group(1))
        dve_ops.OPS.append(op)
        dve_ops.CUSTOM_DVE_SPECS[name] = op.spec
        op.compile("v3")
    return op


def _emb_perm(n_l):
    """Device emb row j -> reference feature row. Rows: 3*n_l sin (l-major),
    3*n_l cos, then x,y,z passthrough."""
    perm = []
    for rep in range(2 * n_l):
        l = rep % n_l
        t = 0 if rep < n_l else 1
        for ax in range(3):
            perm.append(3 + 6 * l + 3 * t + ax)
    perm += [0, 1, 2]
    return np.array(perm)


def _build(weights):
    """Build + compile the Bass program. weights: dict of prepped np arrays."""
    import concourse.bass as bass
    import concourse.tile as tile
    from concourse import bacc, mybir
    from contextlib import ExitStack

    F32 = mybir.dt.float32
    F32R = mybir.dt.float32r
    AF = mybir.ActivationFunctionType
    ALU = mybir.AluOpType

    frac_op = _build_frac_op()

    nc = bacc.Bacc("TRN2", target_bir_lowering=False, debug=False,
                   enable_asserts=False, num_devices=1)

    od_ap = nc.dram_tensor("od", [6, NPC], F32, kind="ExternalInput").ap()
    sv_ap = nc.dram_tensor("sv", [88, 2], F32, kind="ExternalInput").ap()
    bias_ap = nc.dram_tensor("bias", [128, 16], F32, kind="ExternalInput").ap()
    w_aps = {}
    for name in ("w10", "w11", "w12", "w13", "w20a", "w20b", "w21", "w22",
                 "w3e", "w3be", "wc"):
        w_aps[name] = nc.dram_tensor(name, list(weights[name].shape), F32R,
                                     kind="ExternalInput").ap()
    outc_ap = nc.dram_tensor("outc", [3, NPC], F32, kind="ExternalOutput").ap()
    outs_ap = nc.dram_tensor("outs", [1, NPC], F32R, kind="ExternalOutput").ap()

    # bias column indices
    BI = {"b10": 0, "b11": 1, "b12": 2, "b13": 3, "b20": 4, "b21": 5,
          "b22": 6, "b3e": 7, "bc": 8}

    with tile.TileContext(nc) as tc, ExitStack() as ctx:
        cpool = ctx.enter_context(tc.tile_pool(name="const", bufs=1))
        embp = ctx.enter_context(tc.tile_pool(name="emb", bufs=2))
        hp = ctx.enter_context(tc.tile_pool(name="h", bufs=6))
        h9p = ctx.enter_context(tc.tile_pool(name="h9", bufs=3))
        c3p = ctx.enter_context(tc.tile_pool(name="c3", bufs=3))
        psb = ctx.enter_context(tc.tile_pool(name="psb", bufs=4, space="PSUM"))

        wt = {}
        for name, ap in w_aps.items():
            wt[name] = cpool.tile(list(weights[name].shape), F32R, tag=name, name="wt_" + name)
            nc.sync.dma_start(wt[name][:], ap[:])
        svt = cpool.tile([88, 2], F32, tag="sv")
        nc.sync.dma_start(svt[:], sv_ap[:])
        bt = cpool.tile([128, 16], F32, tag="bias")
        nc.sync.dma_start(bt[:], bias_ap[:])

        def bias(name, rows=128):
            return bt[0:rows, BI[name]:BI[name] + 1]

        def relu_evac(eng, dst, src, bname, rows=128):
            if eng == "act":
                nc.scalar.activation(dst, src, AF.Relu, bias=bias(bname, rows))
            else:
                nc.vector.tensor_scalar(dst, src, bias(bname, rows), 0.0,
                                        ALU.add, ALU.max)

        n_batches = NPC // JB
        for b in range(n_batches):
            c0 = b * JB
            # ---- embedding stage (JB-wide) ----
            xb = embp.tile([88, JB], F32, tag="xb")
            src_pos = bass.AP(tensor=od_ap.tensor, offset=c0,
                              ap=[[0, 20], [NPC, 3], [1, JB]])
            nc.sync.dma_start(xb[0:60, :], src_pos)
            src_fill = bass.AP(tensor=od_ap.tensor, offset=c0,
                               ap=[[0, 4], [1, JB]])
            nc.sync.dma_start(xb[60:64, :], src_fill)
            src_dir = bass.AP(tensor=od_ap.tensor, offset=3 * NPC + c0,
                              ap=[[0, 8], [NPC, 3], [1, JB]])
            nc.sync.dma_start(xb[64:88, :], src_dir)

            f = embp.tile([88, JB], F32, tag="f")
            ex = embp.tile([63, JB], F32R, tag="ex")
            ed = embp.tile([27, JB], F32R, tag="ed")
            HB = JB // 2
            for hh in range(2):
                hs = slice(hh * HB, (hh + 1) * HB)
                nc.vector._custom_dve(frac_op, out=f[:, hs], in0=xb[:, hs],
                                      in1=None, s0=svt[:, 0:1], s1=svt[:, 1:2],
                                      imm2=MAGIC)
                nc.scalar.activation(ex[0:60, hs], f[0:60, hs], AF.Sin,
                                     scale=TWO_PI)
                nc.scalar.activation(ed[0:24, hs], f[64:88, hs], AF.Sin,
                                     scale=TWO_PI)
            nc.gpsimd.dma_start(ex[60:63, :], od_ap[0:3, c0:c0 + JB])
            nc.gpsimd.dma_start(ed[24:27, :], od_ap[3:6, c0:c0 + JB])

            nsub = JS // TM
            nchain = JB // JS  # lockstep chains per emb batch

            def mm_layer(wname, ps, rhs_tile, rhs_rows, rhs_col0, extra=None):
                for s in range(nsub):
                    rs = slice(rhs_col0 + s * TM, rhs_col0 + (s + 1) * TM)
                    nc.tensor.matmul(ps[:, s * TM:(s + 1) * TM],
                                     wt[wname][:], rhs_tile[0:rhs_rows, rs],
                                     start=True, stop=not extra)
                    if extra:
                        ew, et, er, ec0 = extra
                        es = slice(ec0 + s * TM, ec0 + (s + 1) * TM)
                        nc.tensor.matmul(ps[:, s * TM:(s + 1) * TM],
                                         wt[ew][:], et[0:er, es],
                                         start=False, stop=True)

            # lockstep chains: layer-by-layer across nchain subtiles; evacs
            # checkerboard across ACT/DVE so same-layer evacs run in parallel
            LAYERS = [
                ("w10", "b10", "ex", None),
                ("w11", "b11", "h", None),
                ("w12", "b12", "h", None),
                ("w13", "b13", "h", None),
                ("w20a", "b20", "h", ("w20b", "ex", 63)),
                ("w21", "b21", "h", None),
                ("w22", "b22", "h", None),
                ("w3e", "b3e", "h", ("w3be", "ed", 27)),
            ]
            hcur = [None] * nchain
            for li, (wname, bname, rhs_kind, extra) in enumerate(LAYERS):
                out_rows = 65 if wname == "w3e" else 128
                for j in range(nchain):
                    jc = j * JS
                    ps = psb.tile([out_rows, JS], mybir.dt.float32, tag="big",
                                  name=f"ps_{wname}_{j}")
                    if rhs_kind == "ex":
                        mm_layer(wname, ps, ex, 63, jc)
                    else:
                        ex_extra = None
                        if extra:
                            ew, ekind, er = extra
                            et = ex if ekind == "ex" else ed
                            ex_extra = (ew, et, er, jc)
                        mm_layer(wname, ps, hcur[j], 128, 0, extra=ex_extra)
                    h = (h9p if out_rows == 65 else hp).tile(
                        [out_rows, JS], F32R,
                        tag="h9" if out_rows == 65 else "h",
                        name=f"h_{wname}_{j}")
                    eng = "act" if (li + j) % 2 == 0 else "dve"
                    relu_evac(eng, h[:], ps[:], bname, out_rows)
                    hcur[j] = h

            for j in range(nchain):
                jc = j * JS
                h9e = hcur[j]
                nc.sync.dma_start(outs_ap[0:1, c0 + jc:c0 + jc + JS],
                                  h9e[64:65, :])
                psc = psb.tile([3, JS], mybir.dt.float32, tag="big",
                               name=f"ps_c3_{j}")
                for s in range(nsub):
                    nc.tensor.matmul(psc[:, s * TM:(s + 1) * TM], wt["wc"][:],
                                     h9e[0:64, s * TM:(s + 1) * TM],
                                     start=True, stop=True)
                c3 = c3p.tile([3, JS], F32, tag="c3", name=f"c3_{j}")
                eng = "act" if j % 2 == 0 else "dve"
                if eng == "act":
                    nc.scalar.activation(c3[:], psc[:], AF.Identity, bias=bias("bc", 3))
                else:
                    nc.vector.tensor_scalar(c3[:], psc[:], bias("bc", 3), None, ALU.add)
                nc.sync.dma_start(outc_ap[:, c0 + jc:c0 + jc + JS], c3[:])

    nc.compile()
    return nc


def _prep(inputs):
    """Host-side weight folding and constant prep (float64 folds, f32 out)."""
    f64 = {k: np.asarray(v, np.float64) for k, v in inputs.items()
           if k not in ("o", "d")}
    perm_x = _emb_perm(L_POS)   # 63
    perm_d = _emb_perm(L_DIR)   # 27

    w10 = f64["w10"][perm_x]
    w20 = f64["w20"]
    w20a, w20b = w20[:128], w20[128:][perm_x]
    w23 = f64["w23"]
    w23c, w23s = w23[:, :128], w23[:, 128]
    b23 = f64["b23"]
    b23c, b23s = b23[:128], b23[128]
    w3 = f64["w3"]
    w3a, w3b = w3[:128], w3[128:][perm_d]

    w3p = w23c @ w3a                       # [128, 64]
    b3p = b23c @ w3a + f64["b3"]           # [64]
    w3e = np.concatenate([w3p, w23s[:, None]], axis=1)        # [128, 65]
    w3be = np.concatenate([w3b, np.zeros((27, 1))], axis=1)   # [27, 65]
    b3e = np.concatenate([b3p, [b23s]])                       # [65]

    W = dict(
        w10=w10, w11=f64["w11"], w12=f64["w12"], w13=f64["w13"],
        w20a=w20a, w20b=w20b, w21=f64["w21"], w22=f64["w22"],
        w3e=w3e, w3be=w3be, wc=f64["w4"],
    )
    # PE reads truncate fp32 operands to FP22 (toward zero): each w*x product
    # is low by ~2*2^-12 in expectation; compensate with a weight pre-scale.
    comp = 1.0 + float(os.environ.get("KERNEL_FP22_COMP", "0"))
    W = {k: np.ascontiguousarray(v * comp, dtype=np.float32) for k, v in W.items()}

    bias = np.zeros((128, 16), np.float32)
    for i, k in enumerate(("b10", "b11", "b12", "b13", "b20", "b21", "b22")):
        bias[:, i] = f64[k].astype(np.float32)
    bias[0:65, 7] = b3e.astype(np.float32)
    bias[0:3, 8] = f64["b4"].astype(np.float32)

    sv = np.zeros((88, 2), np.float32)
    for rep in range(20):
        l = rep % 10
        sv[rep * 3:rep * 3 + 3, 0] = (2.0 ** l) / TWO_PI
        sv[rep * 3:rep * 3 + 3, 1] = 0.0 if rep < 10 else 0.25
    for rep in range(8):
        l = rep % 4
        r0 = 64 + rep * 3
        sv[r0:r0 + 3, 0] = (2.0 ** l) / TWO_PI
        sv[r0:r0 + 3, 1] = 0.0 if rep < 4 else 0.25

    return W, bias, sv


def _run(inputs, trace=False, trace_kwargs=None):
    from concourse.bass_utils import run_bass_kernel_spmd

    W, bias, sv = _prep(inputs)
    key = "nc"
    if key not in _cache:
        _cache[key] = _build(W)
    nc = _cache[key]

    o = np.asarray(inputs["o"], np.float32)
    d = np.asarray(inputs["d"], np.float32)
    oT = np.ascontiguousarray(o.T)  # [3, N]
    dT = np.ascontiguousarray(d.T)

    in_maps = []
    for c in range(NC):
        cs, ce = c * NPC, (c + 1) * NPC
        od = np.ascontiguousarray(np.concatenate([oT[:, cs:ce], dT[:, cs:ce]], axis=0))
        m = dict(od=od, sv=sv, bias=bias)
        for k, v in W.items():
            m[k] = v
        in_maps.append(m)

    kw = {}
    if trace:
        kw["trace"] = True
        if trace_kwargs:
            kw.update(trace_kwargs)
    res = run_bass_kernel_spmd(nc, in_maps, core_ids=list(range(NC)), **kw)

    c_parts, s_parts = [], []
    for cr in range(NC):
        logits = res.results[cr]["outc"]           # [3, NPC]
        sig = res.results[cr]["outs"][0]           # [NPC]
        c_parts.append(0.5 * (1.0 + np.tanh(0.5 * logits.astype(np.float64))))
        s_parts.append(sig)
    c = np.ascontiguousarray(np.concatenate(c_parts, axis=1).T.astype(np.float32))
    sigma = np.concatenate(s_parts).astype(np.float32)
    return (c, sigma), res


def kernel(**inputs):
    (c, sigma), _ = _run(inputs)
    return c, sigma
